# revision 18
# baseline (speedup 1.0000x reference)
"""Trainium2 Bass kernel for nn_Discriminator (RGCN + gated pooling GNN).

Strategy (8 NeuronCores, SPMD):
- Shard the node axis N=4096 into 8 row-blocks of 512 (graph/data parallel).
- The host pre-casts a to fp16 AND pre-transposes it to aT[r, m, n_local]
  (partition-major [R, 128, MT, NP]), so the device does a pure-copy load
  split across the sync+scalar HWDGE queues plus the gpsimd SWDGE queue
  (no on-device cast, no PE transposes).  aT stays SBUF-resident (16 MB)
  and is reused by both RGCN layers.
- msg = sum_r (a[r] @ h) @ w_r in transposed space; B[r]^T accumulates in
  PSUM from per-mt matmuls that fire as soon as each a-chunk lands, so
  layer 0 rides entirely under the a-load.
- One AllGather exchanges h0 between layers (fp16); warmup matmuls and
  the x0-halves of the i/j MLPs run inside the AllGather window to keep
  the PE's HAM clock at 2.4 GHz for the layer-1 burst.  One fp16
  AllReduce combines the per-core segment-sum partials.  The final MLP is
  computed redundantly on every core; output [G, 1] taken from core 0.
"""
import numpy as np

import concourse.bass as bass
import concourse.bacc as bacc
import concourse.tile as tile
import concourse.mybir as mybir
import concourse.bass_utils as bass_utils

P = 8          # cores
T = 5          # atom types
R = 4          # relations
N = 4096       # nodes
G = 512        # graphs
D = 128        # hidden
NP = N // P    # nodes per core (512)
MT = N // 128  # m-tiles (32)
NT = NP // 128  # n-tiles per core (4)
F16 = mybir.dt.float16
F32 = mybir.dt.float32
AF = mybir.ActivationFunctionType

_CACHE = {}

NBLK = 4                 # a-load chunks per relation (1 MB chunks)
CMT = MT // NBLK         # m-tiles per chunk (8)
I8 = mybir.dt.int8


def _build():
    nc = bacc.Bacc("TRN2", target_bir_lowering=False, debug=False,
                   num_devices=P)

    # a transposed+cast on host: aT16[r, p, mt, n] = a[r, c*NP+n, mt*128+p]
    aT16 = nc.dram_tensor("aT16", [R, 128, MT, NP], F16,
                          kind="ExternalInput")
    x0nat = nc.dram_tensor("x0nat", [128, MT, 2 * T], F16,
                           kind="ExternalInput")
    x0To = nc.dram_tensor("x0To", [T, NP], F32, kind="ExternalInput")
    # compact segment one-hot for this core's graph window
    Smc = nc.dram_tensor("Smc", [128, NT, 128], F16, kind="ExternalInput")
    # placement matrices: rank k's window -> global graph columns
    Pm = nc.dram_tensor("Pm", [128, P, G], F16, kind="ExternalInput")
    w0r = nc.dram_tensor("w0r", [R, 2 * T, D], F32, kind="ExternalInput")
    w1r = nc.dram_tensor("w1r", [R, D, D], F32, kind="ExternalInput")
    w0s = nc.dram_tensor("w0s", [T, D], F32, kind="ExternalInput")
    w1s = nc.dram_tensor("w1s", [D, D], F16, kind="ExternalInput")
    iw1a = nc.dram_tensor("iw1a", [T, D], F32, kind="ExternalInput")
    iw1b = nc.dram_tensor("iw1b", [D, D], F16, kind="ExternalInput")
    iw2 = nc.dram_tensor("iw2", [D, D], F16, kind="ExternalInput")
    jw1a = nc.dram_tensor("jw1a", [T, D], F32, kind="ExternalInput")
    jw1b = nc.dram_tensor("jw1b", [D, D], F16, kind="ExternalInput")
    jw2 = nc.dram_tensor("jw2", [D, D], F16, kind="ExternalInput")
    fw1 = nc.dram_tensor("fw1", [D, D], F16, kind="ExternalInput")
    fw2 = nc.dram_tensor("fw2", [D, 1], F16, kind="ExternalInput")
    # bias columns: 0=b0 1=b1 2=ib1 3=ib2 4=jb1 5=jb2 6=fb1
    bias8 = nc.dram_tensor("bias8", [D, 8], F32, kind="ExternalInput")
    fb2v = nc.dram_tensor("fb2v", [1, 1], F32, kind="ExternalInput")
    ident = nc.dram_tensor("ident", [128, 128], F16, kind="ExternalInput")

    outT = nc.dram_tensor("outT", [1, G], F32, kind="ExternalOutput")

    with tile.TileContext(nc) as tc:
        with (
            tc.tile_pool(name="const", bufs=1) as cp,
            tc.tile_pool(name="ares", bufs=1) as ap_,
            tc.tile_pool(name="psBk", bufs=1, space="PSUM") as psBk,
            tc.tile_pool(name="psM", bufs=1, space="PSUM") as psM,
            tc.tile_pool(name="dram", bufs=1, space="DRAM") as dp,
        ):
            # ---- a load: pure fp16 copy, 3 queues, 512 KB chunks ----
            # sync/scalar (HWDGE) carry r0-r2; gpsimd (SWDGE) carries r3.
            aT = [ap_.tile([128, MT, NP], F16, name=f"aT{r}")
                  for r in range(R)]
            for blk in range(NBLK):
                sl = slice(blk * CMT, (blk + 1) * CMT)
                for r in range(R):
                    eng = nc.sync if r < 2 else nc.scalar
                    eng.dma_start(aT[r][:, sl, :], aT16.ap()[r][:, sl, :])

            # ---- constants (small, on whichever queue is free) ----
            ident_sb = cp.tile([128, 128], F16)
            nc.gpsimd.dma_start(ident_sb[:], ident.ap())
            x0n_sb = cp.tile([128, MT, 2 * T], F16)
            nc.gpsimd.dma_start(x0n_sb[:], x0nat.ap())
            x0To_sb = cp.tile([T, NP], F32)
            nc.gpsimd.dma_start(x0To_sb[:], x0To.ap())
            w0r_sb = cp.tile([2 * T, R, D], F32)
            nc.gpsimd.dma_start(w0r_sb[:], w0r.ap().rearrange("r t d -> t r d"))
            w0s_sb = cp.tile([T, D], F32)
            nc.gpsimd.dma_start(w0s_sb[:], w0s.ap())
            bias_sb = cp.tile([D, 8], F32)
            nc.gpsimd.dma_start(bias_sb[:], bias8.ap())
            fb2_sb = cp.tile([1, 1], F32)
            nc.gpsimd.dma_start(fb2_sb[:], fb2v.ap())
            w1r_sb = cp.tile([D, R, D], F32)
            nc.gpsimd.dma_start(w1r_sb[:],
                                w1r.ap().rearrange("r t d -> t r d"))
            w1s_sb = cp.tile([D, D], F16)
            nc.gpsimd.dma_start(w1s_sb[:], w1s.ap())
            iw1a_sb = cp.tile([T, D], F32)
            nc.gpsimd.dma_start(iw1a_sb[:], iw1a.ap())
            iw1b_sb = cp.tile([D, D], F16)
            nc.gpsimd.dma_start(iw1b_sb[:], iw1b.ap())
            iw2_sb = cp.tile([D, D], F16)
            nc.gpsimd.dma_start(iw2_sb[:], iw2.ap())
            jw1a_sb = cp.tile([T, D], F32)
            nc.gpsimd.dma_start(jw1a_sb[:], jw1a.ap())
            jw1b_sb = cp.tile([D, D], F16)
            nc.gpsimd.dma_start(jw1b_sb[:], jw1b.ap())
            jw2_sb = cp.tile([D, D], F16)
            nc.gpsimd.dma_start(jw2_sb[:], jw2.ap())
            fw1_sb = cp.tile([D, D], F16)
            nc.gpsimd.dma_start(fw1_sb[:], fw1.ap())
            fw2_sb = cp.tile([D, 1], F16)
            nc.gpsimd.dma_start(fw2_sb[:], fw2.ap())
            Smc_sb = cp.tile([128, NT, 128], F16)
            nc.gpsimd.dma_start(Smc_sb[:], Smc.ap())
            Pm_sb = cp.tile([128, P, G], F16)
            nc.gpsimd.dma_start(Pm_sb[:], Pm.ap())

            def bias(k):
                return bias_sb[:, k:k + 1]

            with (
                tc.tile_pool(name="work", bufs=1) as wp,
                tc.tile_pool(name="workg", bufs=2) as wg,
                tc.tile_pool(name="bsb", bufs=2) as bp,
                tc.tile_pool(name="psB", bufs=2, space="PSUM") as psB,
                tc.tile_pool(name="psO", bufs=1, space="PSUM") as psO,
            ):
                # ---- pass 0: h0 = tanh(x0 @ w0s + msg0 + b0) ----
                ps_B0 = [psBk.tile([2 * T, NP], F32, name=f"psb{r}",
                                   tag=f"psb{r}") for r in range(R)]
                for blk in range(NBLK):
                    for r in range(R):
                        for j in range(CMT):
                            mt = blk * CMT + j
                            nc.tensor.matmul(
                                ps_B0[r][:], x0n_sb[:, mt, :],
                                aT[r][:, mt, :],
                                start=(mt == 0), stop=(mt == MT - 1))

                ps_msg0 = psM.tile([D, NP], F32, tag="msg", name="ps_msg0")
                nc.tensor.matmul(ps_msg0[:], w0s_sb[:], x0To_sb[:],
                                 start=True, stop=False)
                for r in range(R):
                    B_sb = bp.tile([2 * T, NP], F32, tag="bsb", name="B_sb")
                    nc.vector.tensor_copy(B_sb[:], ps_B0[r][:])
                    nc.tensor.matmul(ps_msg0[:], w0r_sb[:, r, :], B_sb[:],
                                     start=False, stop=(r == R - 1))
                h0To = wp.tile([D, NP], F16)
                nc.scalar.activation(h0To[:], ps_msg0[:], AF.Tanh,
                                     bias=bias(0))
                # int8-quantized copy for the exchange (x127; the 1/127
                # descale is folded into w1r on the host)
                h0q = wp.tile([D, NP], I8)
                nc.scalar.activation(h0q[:], h0To[:], AF.Copy, scale=127.0)

                # ---- AllGather h0 across cores (int8) ----
                ag_in = dp.tile([D, NP], I8)
                ag_out = dp.tile([P, D, NP], I8, addr_space="Shared")
                nc.sync.dma_start(ag_in[:], h0q[:])
                nc.gpsimd.collective_compute(
                    "AllGather", mybir.AluOpType.bypass,
                    replica_groups=[list(range(P))],
                    ins=[ag_in[:]], outs=[ag_out[:]])

                # work that needs no gathered data, inside the AG window:
                # x0-halves of the i/j MLPs + warmup matmuls (keep HAM at
                # 2.4 GHz so the layer-1 burst starts warm).
                ps_ti = psB.tile([D, NP], F32, tag="mlp", name="ps_ti")
                nc.tensor.matmul(ps_ti[:], iw1a_sb[:], x0To_sb[:],
                                 start=True, stop=False)
                ps_tj = psB.tile([D, NP], F32, tag="mlp", name="ps_tj")
                nc.tensor.matmul(ps_tj[:], jw1a_sb[:], x0To_sb[:],
                                 start=True, stop=False)

                h0T8_sb = wp.tile([D, N], I8)
                nc.scalar.dma_start(
                    h0T8_sb[:].rearrange("p (r n) -> p r n", r=P),
                    ag_out[:].rearrange("r p n -> p r n"))
                h0T_sb = wp.tile([D, N], F16)
                nc.vector.tensor_copy(h0T_sb[:], h0T8_sb[:])
                # warm the PE clock during the naturalize: these matmuls
                # depend on the POST-AG h0T_sb, so they fire right before
                # the layer-1 burst rather than right after layer 0.
                ps_wm = psO.tile([D, NP], F32, tag="out", name="ps_wm")
                for _ in range(16):
                    nc.tensor.matmul(ps_wm[:], w1s_sb[:],
                                     h0T_sb[:, 0:NP], start=True, stop=True)
                # naturalize: h0nat[p, mt, d] = h0[mt*128+p, d] (x127 scale)
                h0n_sb = wp.tile([128, MT, D], F16)
                nc.scalar.dma_start(h0n_sb[:], h0T_sb[:], transpose=True)

                # ---- pass 1: h1 = tanh(h0 @ w1s + msg1 + b1) ----
                ps_msg1 = psM.tile([D, NP], F32, tag="msg", name="ps_msg1")
                nc.tensor.matmul(ps_msg1[:], w1s_sb[:], h0To[:],
                                 start=True, stop=False)
                ps_B1 = [psBk.tile([D, NP], F32, name=f"psb1{r}",
                                   tag=f"psb{r}") for r in range(R)]
                for r in range(R):
                    for mt in range(MT):
                        nc.tensor.matmul(
                            ps_B1[r][:], h0n_sb[:, mt, :],
                            aT[r][:, mt, :],
                            start=(mt == 0), stop=(mt == MT - 1))
                    B_sb = bp.tile([D, NP], F32, tag="bsb", name="B1_sb")
                    nc.vector.tensor_copy(B_sb[:], ps_B1[r][:])
                    nc.tensor.matmul(ps_msg1[:], w1r_sb[:, r, :], B_sb[:],
                                     start=False, stop=(r == R - 1))
                h1To = wp.tile([D, NP], F16)
                nc.scalar.activation(h1To[:], ps_msg1[:], AF.Tanh,
                                     bias=bias(1))

                # ---- gated i/j MLPs (x0 halves already accumulated) ----
                nc.tensor.matmul(ps_ti[:], iw1b_sb[:], h1To[:],
                                 start=False, stop=True)
                t_i = wp.tile([D, NP], F16)
                nc.scalar.activation(t_i[:], ps_ti[:], AF.Tanh, bias=bias(2))

                nc.tensor.matmul(ps_tj[:], jw1b_sb[:], h1To[:],
                                 start=False, stop=True)
                t_j = wp.tile([D, NP], F16)
                nc.scalar.activation(t_j[:], ps_tj[:], AF.Tanh, bias=bias(4))

                ps_yj = psB.tile([D, NP], F32, tag="mlp", name="ps_yj")
                nc.tensor.matmul(ps_yj[:], jw2_sb[:], t_j[:], start=True,
                                 stop=True)
                j_sb = wp.tile([D, NP], F16)
                nc.scalar.activation(j_sb[:], ps_yj[:], AF.Tanh,
                                     bias=bias(5))

                ps_yi = psB.tile([D, NP], F32, tag="mlp", name="ps_yi")
                nc.tensor.matmul(ps_yi[:], iw2_sb[:], t_i[:], start=True,
                                 stop=True)
                i_sb = wp.tile([D, NP], F16)
                nc.scalar.activation(i_sb[:], ps_yi[:], AF.Sigmoid,
                                     bias=bias(3))

                gT = wp.tile([D, NP], F16)
                nc.vector.tensor_mul(gT[:], i_sb[:], j_sb[:])
                g_nat = wp.tile([128, NT, D], F16)
                ps_g = psO.tile([128, NT, D], F16, tag="out", name="ps_g")
                for ntt in range(NT):
                    nc.tensor.transpose(ps_g[:, ntt, :],
                                        gT[:, ntt * 128:(ntt + 1) * 128],
                                        ident_sb[:])
                nc.vector.tensor_copy(g_nat[:], ps_g[:])

                # ---- compact local segment sum -> [D, 128] window ----
                ps_plc = psB.tile([D, 128], F32, tag="mlp", name="ps_plc")
                for nt in range(NT):
                    nc.tensor.matmul(
                        ps_plc[:], g_nat[:, nt, :], Smc_sb[:, nt, :],
                        start=(nt == 0), stop=(nt == NT - 1))
                plc_sb = wp.tile([D, 128], F16)
                nc.vector.tensor_copy(plc_sb[:], ps_plc[:])
                ps_pt = psO.tile([128, D], F16, tag="out", name="ps_pt")
                nc.tensor.transpose(ps_pt[:], plc_sb[:], ident_sb[:])
                plT_sb = wp.tile([128, D], F16)
                nc.vector.tensor_copy(plT_sb[:], ps_pt[:])

                # ---- AllGather the compact windows (32 KB per rank) ----
                pg_in = dp.tile([128, D], F16)
                pg_out = dp.tile([P, 128, D], F16, addr_space="Shared")
                nc.sync.dma_start(pg_in[:], plT_sb[:])
                nc.gpsimd.collective_compute(
                    "AllGather", mybir.AluOpType.bypass,
                    replica_groups=[list(range(P))],
                    ins=[pg_in[:]], outs=[pg_out[:]])
                pgs = wg.tile([128, P, D], F16, tag="dg", name="pgs")
                nc.sync.dma_start(pgs[:],
                                  pg_out[:].rearrange("r p d -> p r d"))

                # ---- rebuild pooled[D, G] with placement matmuls ----
                ps_pool = psB.tile([D, G], F32, tag="mlp", name="ps_pool")
                for k in range(P):
                    nc.tensor.matmul(
                        ps_pool[:], pgs[:, k, :], Pm_sb[:, k, :],
                        start=(k == 0), stop=(k == P - 1))
                pooled_t = wg.tile([D, G], F16, tag="dg", name="pooled_t")
                nc.scalar.activation(pooled_t[:], ps_pool[:], AF.Tanh)

                # ---- final MLP ----
                ps_z = psB.tile([D, G], F32, tag="mlp", name="ps_z")
                nc.tensor.matmul(ps_z[:], fw1_sb[:], pooled_t[:], start=True,
                                 stop=True)
                z1_sb = wg.tile([D, G], F16, tag="dg", name="z1_sb")
                nc.scalar.activation(z1_sb[:], ps_z[:], AF.Tanh,
                                     bias=bias(6))

                ps_o = psO.tile([1, G], F32, tag="out", name="ps_o")
                nc.tensor.matmul(ps_o[:], fw2_sb[:], z1_sb[:], start=True,
                                 stop=True)
                out_sb = wp.tile([1, G], F32)
                nc.scalar.activation(out_sb[:], ps_o[:], AF.Identity,
                                     bias=fb2_sb[:, 0:1])
                nc.sync.dma_start(outT.ap(), out_sb[:])

    nc.compile()
    return nc


def _prep_shared(x0, w0s, w0r, b0, w1s, w1r, b1, iw1, ib1, iw2, ib2,
                 jw1, jb1, jw2, jb2, fw1, fb1, fw2, fb2):
    f16 = np.float16
    f32 = np.float32
    x016 = x0.astype(f16)
    x0lo = (x0 - x016.astype(f32)).astype(f16)
    x0hl = np.concatenate([x016, x0lo], axis=1)  # [N, 2T]
    w0r2 = np.concatenate([w0r, w0r], axis=1)    # [R, 2T, D]
    shared = {
        "x0nat": np.ascontiguousarray(
            x0hl.reshape(MT, 128, 2 * T).transpose(1, 0, 2)),
        "w0r": np.ascontiguousarray(w0r2).astype(f32),
        # 1/127 descale for the int8 h0 exchange folded into w1r
        "w1r": np.ascontiguousarray(w1r).astype(f32) / 127.0,
        "w0s": np.ascontiguousarray(w0s).astype(f32),
        "w1s": np.ascontiguousarray(w1s).astype(f16),
        "iw1a": np.ascontiguousarray(iw1[:T]).astype(f32),
        "iw1b": np.ascontiguousarray(iw1[T:]).astype(f16),
        "iw2": np.ascontiguousarray(iw2).astype(f16),
        "jw1a": np.ascontiguousarray(jw1[:T]).astype(f32),
        "jw1b": np.ascontiguousarray(jw1[T:]).astype(f16),
        "jw2": np.ascontiguousarray(jw2).astype(f16),
        "fw1": np.ascontiguousarray(fw1).astype(f16),
        "fw2": np.ascontiguousarray(fw2).astype(f16),
        "bias8": np.stack(
            [b0, b1, ib1, ib2, jb1, jb2, fb1, np.zeros(D, f32)],
            axis=1).astype(f32),
        "fb2v": np.asarray(fb2, f32).reshape(1, 1),
        "ident": np.eye(128, dtype=f16),
    }
    return shared


def kernel(x0, a, segment_ids,
           w0s, w0r, b0, w1s, w1r, b1,
           iw1, ib1, iw2, ib2,
           jw1, jb1, jw2, jb2,
           fw1, fb1, fw2, fb2):
    if "nc" not in _CACHE:
        _CACHE["nc"] = _build()
    nc = _CACHE["nc"]

    x0 = np.asarray(x0, np.float32)
    a = np.asarray(a, np.float32)
    seg = np.asarray(segment_ids)

    shared = _prep_shared(x0, w0s, w0r, b0, w1s, w1r, b1, iw1, ib1, iw2,
                          ib2, jw1, jb1, jw2, jb2, fw1, fb1, fw2, fb2)
    x0T32 = x0.T.astype(np.float32)
    a16 = a.astype(np.float16)

    los = [int(seg[c * NP]) for c in range(P)]
    his = [int(seg[c * NP + NP - 1]) for c in range(P)]
    assert all(his[c] - los[c] + 1 <= 128 for c in range(P)), \
        "per-core graph window exceeds 128"
    # placement matrices (shared across cores: slot k = rank k's window)
    Pmz = np.zeros((128, P, G), np.float16)
    for k in range(P):
        w = his[k] - los[k] + 1
        Pmz[np.arange(w), k, los[k] + np.arange(w)] = 1

    in_maps = []
    for c in range(P):
        sl = slice(c * NP, (c + 1) * NP)
        m = dict(shared)
        # aT16[r, p, mt, n] = a[r, c*NP+n, mt*128+p]
        blk = a16[:, sl, :]                        # [R, NP(n), N(m)]
        blk = blk.transpose(0, 2, 1)               # [R, N(m), NP(n)]
        blk = blk.reshape(R, MT, 128, NP)          # [R, mt, p, n]
        m["aT16"] = np.ascontiguousarray(blk.transpose(0, 2, 1, 3))
        m["x0To"] = np.ascontiguousarray(x0T32[:, sl])
        Sc = (seg[sl, None] ==
              (los[c] + np.arange(128))[None, :]).astype(np.float16)
        m["Smc"] = np.ascontiguousarray(
            Sc.reshape(NT, 128, 128).transpose(1, 0, 2))
        m["Pm"] = Pmz
        in_maps.append(m)

    res = bass_utils.run_bass_kernel_spmd(nc, in_maps,
                                          core_ids=list(range(P)))
    out = np.asarray(res.results[0]["outT"], np.float32).reshape(G, 1)
    return out
